# revision 17
# baseline (speedup 1.0000x reference)
"""ChannelRowAttention Trainium2 kernel (v2).

Full-input contract: kernel(**inputs) takes the complete (8,256,128,128) batch
plus weights, shards batch-wise across 8 NeuronCores (one image per core), and
returns the full (8,256,128,128) output.

Per-core plan (x_img = (256,128,128), fp16 on chip; residual path fp16):
  x resident in SBUF (fp16, 64KB/partition), loaded once in 8 chunks.

  pass 1, per 4-row block (software-pipelined one block ahead on PE):
    kq    = [Wk|Wq]^T . x_rows       (PE, M=128: psum parts 0:64=k, 64:128=q)
    kq -> SBUF fp16 (ACT), k replicated to partitions 64:128 via SBUF DMA
    attT_r = k_r^T q_r               (PE, K=64 @ base 64) -> attT[j,i] directly
                                     (no PE transposes anywhere)
    exp on ACT (fp32 psum -> bf16 SBUF, no max-subtraction; |score|<40 and
    bf16 holds e^40)
    den  = ones^T attT_e             (PE, M=1)  -> 1/den on DVE (approx recip)
    inv broadcast to 128 partitions  (GPSIMD partition_broadcast)
    vT_r  = x_row^T . Wv^T           (PE, N=256 per row)
    out_r = vT^T . attT_e            (PE; unnormalized, fp32 psum)
    normalization by inv[i] is folded into the psum->SBUF output copies
    (DVE tensor_tensor_reduce), whose accum_out also chains the per-channel
    sum stat; running max stat on GPSIMD.
  gate  = sigmoid(W2.relu(W1.avg) + W2.relu(W1.max)): tiny fp32 PE matmuls
  pass 2, per block: final = (out * (gama*gate[c])) + x -> DRAM fp16
  (host casts back to fp32)
"""

import numpy as np
from contextlib import ExitStack

import concourse.bass as bass
from concourse import bacc
import concourse.tile as tile
from concourse import mybir
from concourse.bass_utils import run_bass_kernel_spmd

F32 = mybir.dt.float32
F16 = mybir.dt.float16
BF16 = mybir.dt.bfloat16

N, C, H, W = 8, 256, 128, 128
QK = 64
HID = 16          # SE hidden dim = C // 16
NCORES = 8
RB = 4            # rows per block
NBLK = H // RB    # 32
INV_HW = 1.0 / float(H * W)

AX = mybir.AxisListType
OP = mybir.AluOpType
AF = mybir.ActivationFunctionType

# feature flags (bisection / perf toggles)
USE_APPROX_RECIP = True   # custom-DVE fast reciprocal
USE_TTR = False            # fused normalize+copy+sum via tensor_tensor_reduce
USE_K2_DMA = True         # replicate k via SBUF->SBUF DMA (else extra matmuls)


def _body(ctx: ExitStack, tc: "tile.TileContext", xh_d, wqk_d, wv_d,
          w1_d, w2_d, gama_d, y_d):
    nc = tc.nc

    const = ctx.enter_context(tc.tile_pool(name="const", bufs=1))
    stats = ctx.enter_context(tc.tile_pool(name="stats", bufs=1))
    xpool = ctx.enter_context(tc.tile_pool(name="xpool", bufs=1))
    opool = ctx.enter_context(tc.tile_pool(name="opool", bufs=1))
    work = ctx.enter_context(tc.tile_pool(name="work", bufs=2))
    finpool = ctx.enter_context(tc.tile_pool(name="fin", bufs=4))
    psK = ctx.enter_context(
        tc.tile_pool(name="psK", bufs=2 if USE_K2_DMA else 1, space="PSUM"))
    psT = ctx.enter_context(tc.tile_pool(name="psT", bufs=1, space="PSUM"))
    psD = ctx.enter_context(tc.tile_pool(name="psD", bufs=1, space="PSUM"))
    psV = ctx.enter_context(tc.tile_pool(name="psV", bufs=1, space="PSUM"))
    psO = ctx.enter_context(tc.tile_pool(name="psO", bufs=1, space="PSUM"))

    # ---- constants -------------------------------------------------------
    wqk_sb = const.tile([128, 2, 128], F16)
    nc.sync.dma_start(out=wqk_sb, in_=wqk_d[:, :].rearrange("(kc p) m -> p kc m", p=128))
    wv_sb = const.tile([128, 2, C], F16)
    nc.sync.dma_start(out=wv_sb, in_=wv_d[:, :].rearrange("(kc p) m -> p kc m", p=128))
    w1_sb = const.tile([128, 2, HID], F32)
    nc.sync.dma_start(out=w1_sb, in_=w1_d[:, :].rearrange("(kc p) m -> p kc m", p=128))
    w2_sb = const.tile([HID, 2, 128], F32)
    nc.sync.dma_start(out=w2_sb, in_=w2_d[:, :].rearrange("k (mc m) -> k mc m", m=128))
    gama_sb = const.tile([128, 1], F32)
    nc.sync.dma_start(out=gama_sb, in_=gama_d[:, :].to_broadcast([128, 1]))
    ones_sb = const.tile([128, 128], BF16)
    nc.vector.memset(ones_sb, 1.0)
    gscale = const.tile([128, 2], F32)      # gama * sigmoid(gate), filled later

    # ---- resident image + attention output ------------------------------
    x_sb = xpool.tile([128, 2, H, W], F16)
    CHUNK = 16
    for i in range(H // CHUNK):
        nc.sync.dma_start(
            out=x_sb[:, :, i * CHUNK:(i + 1) * CHUNK, :],
            in_=xh_d[:, i * CHUNK:(i + 1) * CHUNK, :]
                .rearrange("(kc p) h w -> p kc h w", p=128),
        )
    ob_all = opool.tile([128, 2, H, W], F16)

    # stats: chained per-channel sums (ping-pong) and running max (ping-pong)
    s0 = [stats.tile([128, 1], F32, name=f"s0{i}") for i in (0, 1)]
    s1 = [stats.tile([128, 1], F32, name=f"s1{i}") for i in (0, 1)]
    sums_acc = stats.tile([128, 2, NBLK], F32)
    if not USE_TTR:
        nc.vector.memset(sums_acc, 0.0)
    mxa = stats.tile([128, 2, RB, W], F16)
    nc.vector.memset(mxa, -60000.0)
    mxb = stats.tile([128, 2, RB, W], F16)

    # ---- pass 1 (PE pipelined one block ahead) ---------------------------
    kq_sbs = [None] * NBLK
    k2_sbs = [None] * NBLK

    def stage_kq(b):
        h0 = b * RB
        xr = x_sb[:, :, h0:h0 + RB, :]
        kq_ps = psK.tile([128, RB, W], F32, tag="kq")
        for kc in (0, 1):
            nc.tensor.matmul(
                out=kq_ps[:, :, :].rearrange("p r w -> p (r w)"),
                lhsT=wqk_sb[:, kc, :],
                rhs=xr[:, kc, :, :].rearrange("p r w -> p (r w)"),
                start=(kc == 0), stop=(kc == 1),
            )
        kq_sb = work.tile([128, RB, W], F16, tag="kq_sb")
        nc.scalar.copy(out=kq_sb, in_=kq_ps)
        # replicate k (parts 0:64) to parts 64:128 so att operands share base
        k2_sb = work.tile([128, RB, W], F16, tag="k2_sb")
        if USE_K2_DMA:
            nc.sync.dma_start(out=k2_sb[64:128, :, :], in_=kq_sb[0:64, :, :])
        else:
            k2_ps = psK.tile([128, RB, W], F32, tag="kq2")
            for kc in (0, 1):
                nc.tensor.matmul(
                    out=k2_ps[64:128, :, :].rearrange("p r w -> p (r w)"),
                    lhsT=wqk_sb[:, kc, 0:64],
                    rhs=xr[:, kc, :, :].rearrange("p r w -> p (r w)"),
                    start=(kc == 0), stop=(kc == 1),
                )
            nc.scalar.copy(out=k2_sb[64:128, :, :], in_=k2_ps[64:128, :, :])
        kq_sbs[b] = kq_sb
        k2_sbs[b] = k2_sb

    def stage_main(b):
        h0 = b * RB
        kq_sb, k2_sb = kq_sbs[b], k2_sbs[b]

        # attT[j, i] per row (K=64 at base partition 64)
        attT_ps = psT.tile([128, RB, W], F32, tag="attT")
        for r in range(RB):
            nc.tensor.matmul(
                out=attT_ps[:, r, :],
                lhsT=k2_sb[64:128, r, :],
                rhs=kq_sb[64:128, r, :],
                start=True, stop=True,
            )
        attT_e = work.tile([128, RB, W], BF16, tag="attT_e")
        nc.scalar.activation(out=attT_e, in_=attT_ps, func=AF.Exp)

        # vT per row (w on partitions, c on free)
        vt_ps = psV.tile([128, RB, C], F32, tag="vt")
        for r in range(RB):
            for kc in (0, 1):
                nc.tensor.matmul(
                    out=vt_ps[:, r, :],
                    lhsT=x_sb[:, kc, h0 + r, :],
                    rhs=wv_sb[:, kc, :],
                    start=(kc == 0), stop=(kc == 1),
                )

        # softmax denominator, broadcast across partitions in one matmul:
        # ones[128,128]^T @ attT_e -> every psum partition holds den[(r,i)]
        den_ps = psD.tile([128, RB * W], F32, tag="den")
        nc.tensor.matmul(
            out=den_ps,
            lhsT=ones_sb,
            rhs=attT_e[:, :, :].rearrange("p r w -> p (r w)"),
            start=True, stop=True,
        )
        inv_b = work.tile([128, RB, W], F32, tag="inv_b")
        inv_flat = inv_b[:, :, :].rearrange("p r w -> p (r w)")
        if USE_APPROX_RECIP:
            nc.vector.reciprocal_approx_fast(out=inv_flat, in_=den_ps)
        else:
            nc.vector.reciprocal(out=inv_flat, in_=den_ps)

        vt_sb = work.tile([128, RB, C], BF16, tag="vt_sb")
        nc.scalar.copy(out=vt_sb[:, 0:2, :], in_=vt_ps[:, 0:2, :])
        nc.scalar.copy(out=vt_sb[:, 2:4, :], in_=vt_ps[:, 2:4, :])

        if USE_TTR:
            att_mm_rhs = attT_e
        else:
            # normalize attT explicitly; out-copies are then plain copies
            attT_n = work.tile([128, RB, W], F16, tag="attT_n")
            nc.vector.tensor_tensor(
                out=attT_n, in0=attT_e, in1=inv_b, op=OP.mult)
            att_mm_rhs = attT_n

        # out = vT^T @ attT -> (c, i)
        out_ps = psO.tile([128, 2, RB, W], F32, tag="out")
        for r in range(RB):
            for ch in (0, 1):
                nc.tensor.matmul(
                    out=out_ps[:, ch, r, :],
                    lhsT=vt_sb[:, r, 128 * ch:128 * (ch + 1)],
                    rhs=att_mm_rhs[:, r, :],
                    start=True, stop=True,
                )

        if USE_TTR:
            # psum -> resident SBUF fp16, normalizing by inv[i]; accum_out
            # chains the per-channel sum stat across blocks
            for ch in (0, 1):
                chain = s0 if ch == 0 else s1
                prev = 0.0 if b == 0 else chain[(b - 1) % 2]
                nc.vector.tensor_tensor_reduce(
                    out=ob_all[:, ch, h0:h0 + RB, :],
                    in0=out_ps[:, ch],
                    in1=inv_b,
                    scale=1.0,
                    scalar=prev,
                    op0=OP.mult,
                    op1=OP.add,
                    accum_out=chain[b % 2],
                )
        else:
            nc.vector.tensor_scalar(
                out=ob_all[:, 0, h0:h0 + RB, :], in0=out_ps[:, 0],
                scalar1=1.0, scalar2=0.0, op0=OP.mult, op1=OP.add,
                accum_out=sums_acc[:, 0, b:b + 1])
            nc.scalar.activation(
                out=ob_all[:, 1, h0:h0 + RB, :], in_=out_ps[:, 1],
                func=AF.Copy, accum_out=sums_acc[:, 1, b:b + 1])
        # running max stat (DVE fp16 2x mode), ping-pong accumulators
        src, dst = (mxa, mxb) if b % 2 == 0 else (mxb, mxa)
        nc.vector.tensor_tensor(
            out=dst, in0=src, in1=ob_all[:, :, h0:h0 + RB, :], op=OP.max)

    for b in range(NBLK + 1):
        if b < NBLK:
            stage_kq(b)
        if b >= 1:
            stage_main(b - 1)

    # ---- gate ------------------------------------------------------------
    mxfin = mxa if NBLK % 2 == 0 else mxb

    mx = stats.tile([128, 2], F32)
    nc.vector.tensor_reduce(out=mx, in_=mxfin, axis=AX.XY, op=OP.max)

    mlp_in = stats.tile([128, 2, 2], F32)
    if USE_TTR:
        sfin = [s0[(NBLK - 1) % 2], s1[(NBLK - 1) % 2]]
        for ch in (0, 1):
            nc.vector.tensor_scalar_mul(
                out=mlp_in[:, ch, 0:1], in0=sfin[ch], scalar1=INV_HW)
    else:
        sums = stats.tile([128, 2], F32)
        nc.vector.tensor_reduce(out=sums, in_=sums_acc, axis=AX.X, op=OP.add)
        nc.vector.tensor_scalar_mul(
            out=mlp_in[:, :, 0], in0=sums, scalar1=INV_HW)
    nc.vector.tensor_copy(out=mlp_in[:, :, 1], in_=mx)

    h_ps = psD.tile([HID, 2], F32, tag="den")
    for kc in (0, 1):
        nc.tensor.matmul(
            out=h_ps,
            lhsT=w1_sb[:, kc, :],
            rhs=mlp_in[:, kc, :],
            start=(kc == 0), stop=(kc == 1),
        )
    hr = stats.tile([HID, 2], F32)
    nc.vector.tensor_scalar_max(out=hr, in0=h_ps, scalar1=0.0)
    g_ps = psD.tile([128, 2, 2], F32, tag="den")
    for mc in (0, 1):
        nc.tensor.matmul(
            out=g_ps[:, mc, :],
            lhsT=w2_sb[:, mc, :],
            rhs=hr,
            start=True, stop=True,
        )
    zt = stats.tile([128, 2], F32)
    nc.vector.tensor_reduce(out=zt, in_=g_ps, axis=AX.X, op=OP.add)
    th = stats.tile([128, 2], F32)
    nc.scalar.activation(out=th, in_=zt, func=AF.Tanh, scale=0.5)
    u = stats.tile([128, 2], F32)
    nc.vector.tensor_scalar_add(out=u, in0=th, scalar1=1.0)
    # gscale = gama * sigmoid(z) = gama * 0.5 * (1 + tanh(z/2))
    nc.vector.tensor_scalar(
        out=gscale, in0=u, scalar1=gama_sb, scalar2=0.5, op0=OP.mult, op1=OP.mult)

    # ---- pass 2: final = out*gscale[c] + x -> DRAM (fp16) ----------------
    for b in range(NBLK):
        h0 = b * RB
        fin = finpool.tile([128, 2, RB, W], F16, tag="fin")
        nc.vector.scalar_tensor_tensor(
            out=fin[:, 0], in0=ob_all[:, 0, h0:h0 + RB, :],
            scalar=gscale[:, 0:1], in1=x_sb[:, 0, h0:h0 + RB, :],
            op0=OP.mult, op1=OP.add)
        nc.vector.scalar_tensor_tensor(
            out=fin[:, 1], in0=ob_all[:, 1, h0:h0 + RB, :],
            scalar=gscale[:, 1:2], in1=x_sb[:, 1, h0:h0 + RB, :],
            op0=OP.mult, op1=OP.add)
        nc.sync.dma_start(
            out=y_d[:, h0:h0 + RB, :].rearrange("(kc p) h w -> p kc h w", p=128),
            in_=fin,
        )


def build_nc() -> bass.Bass:
    nc = bacc.Bacc()
    xh_d = nc.dram_tensor("xh", [C, H, W], F16, kind="ExternalInput")
    wqk_d = nc.dram_tensor("wqkT", [C, 128], F16, kind="ExternalInput")
    wv_d = nc.dram_tensor("wvT", [C, C], F16, kind="ExternalInput")
    w1_d = nc.dram_tensor("w1T", [C, HID], F32, kind="ExternalInput")
    w2_d = nc.dram_tensor("w2T", [HID, C], F32, kind="ExternalInput")
    gama_d = nc.dram_tensor("gama", [1, 1], F32, kind="ExternalInput")
    y_d = nc.dram_tensor("out", [C, H, W], F16, kind="ExternalOutput")

    with tile.TileContext(nc) as tc:
        with ExitStack() as ctx:
            _body(ctx, tc, xh_d[:, :, :], wqk_d[:, :],
                  wv_d[:, :], w1_d[:, :], w2_d[:, :], gama_d[:, :],
                  y_d[:, :, :])
    nc.compile()
    return nc


_NC_CACHE = {}


def _get_nc():
    if "nc" not in _NC_CACHE:
        _NC_CACHE["nc"] = build_nc()
    return _NC_CACHE["nc"]


def _make_in_maps(x, Wq, Wk, Wv, W1, W2, gama):
    wqkT = np.ascontiguousarray(
        np.concatenate([Wk, Wq], axis=0).T.astype(np.float16))
    wvT = np.ascontiguousarray(Wv.T.astype(np.float16))
    w1T = np.ascontiguousarray(W1.T.astype(np.float32))
    w2T = np.ascontiguousarray(W2.T.astype(np.float32))
    g = np.asarray(gama, dtype=np.float32).reshape(1, 1)
    maps = []
    for i in range(NCORES):
        maps.append({
            "xh": np.ascontiguousarray(x[i].astype(np.float16)),
            "wqkT": wqkT, "wvT": wvT, "w1T": w1T, "w2T": w2T, "gama": g,
        })
    return maps


def run(x, Wq, Wk, Wv, W1, W2, gama, trace=False):
    nc = _get_nc()
    in_maps = _make_in_maps(x, Wq, Wk, Wv, W1, W2, gama)
    res = run_bass_kernel_spmd(nc, in_maps, core_ids=list(range(NCORES)),
                               trace=trace)
    y = np.stack([res.results[i]["out"].astype(np.float32)
                  for i in range(NCORES)], axis=0)
    return y, res


def kernel(x, Wq, Wk, Wv, W1, W2, gama):
    x = np.asarray(x); Wq = np.asarray(Wq); Wk = np.asarray(Wk)
    Wv = np.asarray(Wv); W1 = np.asarray(W1); W2 = np.asarray(W2)
    gama = np.asarray(gama)
    y, _ = run(x, Wq, Wk, Wv, W1, W2, gama, trace=False)
    return y.astype(np.float32)


# revision 18
# speedup vs baseline: 1.0533x; 1.0533x over previous
"""ChannelRowAttention Trainium2 kernel (v2).

Full-input contract: kernel(**inputs) takes the complete (8,256,128,128) batch
plus weights, shards batch-wise across 8 NeuronCores (one image per core), and
returns the full (8,256,128,128) output.

Per-core plan (x_img = (256,128,128), fp16 on chip; residual path fp16):
  x resident in SBUF (fp16, 64KB/partition), loaded once in 8 chunks.

  pass 1, per 4-row block (software-pipelined one block ahead on PE):
    kq    = [Wk|Wq]^T . x_rows       (PE, M=128: psum parts 0:64=k, 64:128=q)
    kq -> SBUF fp16 (ACT), k replicated to partitions 64:128 via SBUF DMA
    attT_r = k_r^T q_r               (PE, K=64 @ base 64) -> attT[j,i] directly
                                     (no PE transposes anywhere)
    exp on ACT (fp32 psum -> bf16 SBUF, no max-subtraction; |score|<40 and
    bf16 holds e^40)
    den  = ones^T attT_e             (PE, M=1)  -> 1/den on DVE (approx recip)
    inv broadcast to 128 partitions  (GPSIMD partition_broadcast)
    vT_r  = x_row^T . Wv^T           (PE, N=256 per row)
    out_r = vT^T . attT_e            (PE; unnormalized, fp32 psum)
    normalization by inv[i] is folded into the psum->SBUF output copies
    (DVE tensor_tensor_reduce), whose accum_out also chains the per-channel
    sum stat; running max stat on GPSIMD.
  gate  = sigmoid(W2.relu(W1.avg) + W2.relu(W1.max)): tiny fp32 PE matmuls
  pass 2, per block: final = (out * (gama*gate[c])) + x -> DRAM fp16
  (host casts back to fp32)
"""

import numpy as np
from contextlib import ExitStack

import concourse.bass as bass
from concourse import bacc
import concourse.tile as tile
from concourse import mybir
from concourse.bass_utils import run_bass_kernel_spmd

F32 = mybir.dt.float32
F16 = mybir.dt.float16
BF16 = mybir.dt.bfloat16

N, C, H, W = 8, 256, 128, 128
QK = 64
HID = 16          # SE hidden dim = C // 16
NCORES = 8
RB = 4            # rows per block
NBLK = H // RB    # 32
INV_HW = 1.0 / float(H * W)

AX = mybir.AxisListType
OP = mybir.AluOpType
AF = mybir.ActivationFunctionType

# feature flags (bisection / perf toggles)
USE_APPROX_RECIP = True   # custom-DVE fast reciprocal
USE_TTR = False            # fused normalize+copy+sum via tensor_tensor_reduce
USE_K2_DMA = False         # replicate k via SBUF->SBUF DMA (else extra matmuls)


def _body(ctx: ExitStack, tc: "tile.TileContext", xh_d, wqk_d, wv_d,
          w1_d, w2_d, gama_d, y_d):
    nc = tc.nc

    const = ctx.enter_context(tc.tile_pool(name="const", bufs=1))
    stats = ctx.enter_context(tc.tile_pool(name="stats", bufs=1))
    xpool = ctx.enter_context(tc.tile_pool(name="xpool", bufs=1))
    opool = ctx.enter_context(tc.tile_pool(name="opool", bufs=1))
    work = ctx.enter_context(tc.tile_pool(name="work", bufs=2))
    finpool = ctx.enter_context(tc.tile_pool(name="fin", bufs=4))
    psK = ctx.enter_context(
        tc.tile_pool(name="psK", bufs=2 if USE_K2_DMA else 1, space="PSUM"))
    psT = ctx.enter_context(tc.tile_pool(name="psT", bufs=1, space="PSUM"))
    psD = ctx.enter_context(tc.tile_pool(name="psD", bufs=1, space="PSUM"))
    psV = ctx.enter_context(tc.tile_pool(name="psV", bufs=1, space="PSUM"))
    psO = ctx.enter_context(tc.tile_pool(name="psO", bufs=1, space="PSUM"))

    # ---- constants -------------------------------------------------------
    wqk_sb = const.tile([128, 2, 128], F16)
    nc.sync.dma_start(out=wqk_sb, in_=wqk_d[:, :].rearrange("(kc p) m -> p kc m", p=128))
    wv_sb = const.tile([128, 2, C], F16)
    nc.sync.dma_start(out=wv_sb, in_=wv_d[:, :].rearrange("(kc p) m -> p kc m", p=128))
    w1_sb = const.tile([128, 2, HID], F32)
    nc.sync.dma_start(out=w1_sb, in_=w1_d[:, :].rearrange("(kc p) m -> p kc m", p=128))
    w2_sb = const.tile([HID, 2, 128], F32)
    nc.sync.dma_start(out=w2_sb, in_=w2_d[:, :].rearrange("k (mc m) -> k mc m", m=128))
    gama_sb = const.tile([128, 1], F32)
    nc.sync.dma_start(out=gama_sb, in_=gama_d[:, :].to_broadcast([128, 1]))
    ones_sb = const.tile([128, 128], BF16)
    nc.vector.memset(ones_sb, 1.0)
    gscale = const.tile([128, 2], F32)      # gama * sigmoid(gate), filled later

    # ---- resident image + attention output ------------------------------
    x_sb = xpool.tile([128, 2, H, W], F16)
    CHUNK = 16
    for i in range(H // CHUNK):
        nc.sync.dma_start(
            out=x_sb[:, :, i * CHUNK:(i + 1) * CHUNK, :],
            in_=xh_d[:, i * CHUNK:(i + 1) * CHUNK, :]
                .rearrange("(kc p) h w -> p kc h w", p=128),
        )
    ob_all = opool.tile([128, 2, H, W], F16)

    # stats: chained per-channel sums (ping-pong) and running max (ping-pong)
    s0 = [stats.tile([128, 1], F32, name=f"s0{i}") for i in (0, 1)]
    s1 = [stats.tile([128, 1], F32, name=f"s1{i}") for i in (0, 1)]
    sums_acc = stats.tile([128, 2, NBLK], F32)
    if not USE_TTR:
        nc.vector.memset(sums_acc, 0.0)
    mxa = stats.tile([128, 2, RB, W], F16)
    nc.vector.memset(mxa, -60000.0)
    mxb = stats.tile([128, 2, RB, W], F16)

    # ---- pass 1 (PE pipelined one block ahead) ---------------------------
    kq_sbs = [None] * NBLK
    k2_sbs = [None] * NBLK

    def stage_kq(b):
        h0 = b * RB
        xr = x_sb[:, :, h0:h0 + RB, :]
        kq_ps = psK.tile([128, RB, W], F32, tag="kq")
        for kc in (0, 1):
            nc.tensor.matmul(
                out=kq_ps[:, :, :].rearrange("p r w -> p (r w)"),
                lhsT=wqk_sb[:, kc, :],
                rhs=xr[:, kc, :, :].rearrange("p r w -> p (r w)"),
                start=(kc == 0), stop=(kc == 1),
            )
        kq_sb = work.tile([128, RB, W], F16, tag="kq_sb")
        nc.scalar.copy(out=kq_sb, in_=kq_ps)
        # replicate k (parts 0:64) to parts 64:128 so att operands share base
        k2_sb = work.tile([128, RB, W], F16, tag="k2_sb")
        if USE_K2_DMA:
            nc.sync.dma_start(out=k2_sb[64:128, :, :], in_=kq_sb[0:64, :, :])
        else:
            k2_ps = psK.tile([128, RB, W], F32, tag="kq2")
            for kc in (0, 1):
                nc.tensor.matmul(
                    out=k2_ps[64:128, :, :].rearrange("p r w -> p (r w)"),
                    lhsT=wqk_sb[:, kc, 0:64],
                    rhs=xr[:, kc, :, :].rearrange("p r w -> p (r w)"),
                    start=(kc == 0), stop=(kc == 1),
                )
            nc.scalar.copy(out=k2_sb[64:128, :, :], in_=k2_ps[64:128, :, :])
        kq_sbs[b] = kq_sb
        k2_sbs[b] = k2_sb

    def stage_main(b):
        h0 = b * RB
        kq_sb, k2_sb = kq_sbs[b], k2_sbs[b]

        # attT[j, i] per row (K=64 at base partition 64)
        attT_ps = psT.tile([128, RB, W], F32, tag="attT")
        for r in range(RB):
            nc.tensor.matmul(
                out=attT_ps[:, r, :],
                lhsT=k2_sb[64:128, r, :],
                rhs=kq_sb[64:128, r, :],
                start=True, stop=True,
            )
        attT_e = work.tile([128, RB, W], BF16, tag="attT_e")
        nc.scalar.activation(out=attT_e, in_=attT_ps, func=AF.Exp)

        # vT per row (w on partitions, c on free)
        vt_ps = psV.tile([128, RB, C], F32, tag="vt")
        for r in range(RB):
            for kc in (0, 1):
                nc.tensor.matmul(
                    out=vt_ps[:, r, :],
                    lhsT=x_sb[:, kc, h0 + r, :],
                    rhs=wv_sb[:, kc, :],
                    start=(kc == 0), stop=(kc == 1),
                )

        # softmax denominator, broadcast across partitions in one matmul:
        # ones[128,128]^T @ attT_e -> every psum partition holds den[(r,i)]
        den_ps = psD.tile([128, RB * W], F32, tag="den")
        nc.tensor.matmul(
            out=den_ps,
            lhsT=ones_sb,
            rhs=attT_e[:, :, :].rearrange("p r w -> p (r w)"),
            start=True, stop=True,
        )
        inv_b = work.tile([128, RB, W], F32, tag="inv_b")
        inv_flat = inv_b[:, :, :].rearrange("p r w -> p (r w)")
        if USE_APPROX_RECIP:
            nc.vector.reciprocal_approx_fast(out=inv_flat, in_=den_ps)
        else:
            nc.vector.reciprocal(out=inv_flat, in_=den_ps)

        vt_sb = work.tile([128, RB, C], BF16, tag="vt_sb")
        nc.scalar.copy(out=vt_sb[:, 0:2, :], in_=vt_ps[:, 0:2, :])
        nc.scalar.copy(out=vt_sb[:, 2:4, :], in_=vt_ps[:, 2:4, :])

        if USE_TTR:
            att_mm_rhs = attT_e
        else:
            # normalize attT explicitly; out-copies are then plain copies
            attT_n = work.tile([128, RB, W], F16, tag="attT_n")
            nc.vector.tensor_tensor(
                out=attT_n, in0=attT_e, in1=inv_b, op=OP.mult)
            att_mm_rhs = attT_n

        # out = vT^T @ attT -> (c, i)
        out_ps = psO.tile([128, 2, RB, W], F32, tag="out")
        for r in range(RB):
            for ch in (0, 1):
                nc.tensor.matmul(
                    out=out_ps[:, ch, r, :],
                    lhsT=vt_sb[:, r, 128 * ch:128 * (ch + 1)],
                    rhs=att_mm_rhs[:, r, :],
                    start=True, stop=True,
                )

        if USE_TTR:
            # psum -> resident SBUF fp16, normalizing by inv[i]; accum_out
            # chains the per-channel sum stat across blocks
            for ch in (0, 1):
                chain = s0 if ch == 0 else s1
                prev = 0.0 if b == 0 else chain[(b - 1) % 2]
                nc.vector.tensor_tensor_reduce(
                    out=ob_all[:, ch, h0:h0 + RB, :],
                    in0=out_ps[:, ch],
                    in1=inv_b,
                    scale=1.0,
                    scalar=prev,
                    op0=OP.mult,
                    op1=OP.add,
                    accum_out=chain[b % 2],
                )
        else:
            nc.vector.tensor_scalar(
                out=ob_all[:, 0, h0:h0 + RB, :], in0=out_ps[:, 0],
                scalar1=1.0, scalar2=0.0, op0=OP.mult, op1=OP.add,
                accum_out=sums_acc[:, 0, b:b + 1])
            nc.scalar.activation(
                out=ob_all[:, 1, h0:h0 + RB, :], in_=out_ps[:, 1],
                func=AF.Copy, accum_out=sums_acc[:, 1, b:b + 1])
        # running max stat (DVE fp16 2x mode), ping-pong accumulators
        src, dst = (mxa, mxb) if b % 2 == 0 else (mxb, mxa)
        nc.vector.tensor_tensor(
            out=dst, in0=src, in1=ob_all[:, :, h0:h0 + RB, :], op=OP.max)

    for b in range(NBLK + 1):
        if b < NBLK:
            stage_kq(b)
        if b >= 1:
            stage_main(b - 1)

    # ---- gate ------------------------------------------------------------
    mxfin = mxa if NBLK % 2 == 0 else mxb

    mx = stats.tile([128, 2], F32)
    nc.vector.tensor_reduce(out=mx, in_=mxfin, axis=AX.XY, op=OP.max)

    mlp_in = stats.tile([128, 2, 2], F32)
    if USE_TTR:
        sfin = [s0[(NBLK - 1) % 2], s1[(NBLK - 1) % 2]]
        for ch in (0, 1):
            nc.vector.tensor_scalar_mul(
                out=mlp_in[:, ch, 0:1], in0=sfin[ch], scalar1=INV_HW)
    else:
        sums = stats.tile([128, 2], F32)
        nc.vector.tensor_reduce(out=sums, in_=sums_acc, axis=AX.X, op=OP.add)
        nc.vector.tensor_scalar_mul(
            out=mlp_in[:, :, 0], in0=sums, scalar1=INV_HW)
    nc.vector.tensor_copy(out=mlp_in[:, :, 1], in_=mx)

    h_ps = psD.tile([HID, 2], F32, tag="den")
    for kc in (0, 1):
        nc.tensor.matmul(
            out=h_ps,
            lhsT=w1_sb[:, kc, :],
            rhs=mlp_in[:, kc, :],
            start=(kc == 0), stop=(kc == 1),
        )
    hr = stats.tile([HID, 2], F32)
    nc.vector.tensor_scalar_max(out=hr, in0=h_ps, scalar1=0.0)
    g_ps = psD.tile([128, 2, 2], F32, tag="den")
    for mc in (0, 1):
        nc.tensor.matmul(
            out=g_ps[:, mc, :],
            lhsT=w2_sb[:, mc, :],
            rhs=hr,
            start=True, stop=True,
        )
    zt = stats.tile([128, 2], F32)
    nc.vector.tensor_reduce(out=zt, in_=g_ps, axis=AX.X, op=OP.add)
    th = stats.tile([128, 2], F32)
    nc.scalar.activation(out=th, in_=zt, func=AF.Tanh, scale=0.5)
    u = stats.tile([128, 2], F32)
    nc.vector.tensor_scalar_add(out=u, in0=th, scalar1=1.0)
    # gscale = gama * sigmoid(z) = gama * 0.5 * (1 + tanh(z/2))
    nc.vector.tensor_scalar(
        out=gscale, in0=u, scalar1=gama_sb, scalar2=0.5, op0=OP.mult, op1=OP.mult)

    # ---- pass 2: final = out*gscale[c] + x -> DRAM (fp16) ----------------
    for b in range(NBLK):
        h0 = b * RB
        fin = finpool.tile([128, 2, RB, W], F16, tag="fin")
        nc.vector.scalar_tensor_tensor(
            out=fin[:, 0], in0=ob_all[:, 0, h0:h0 + RB, :],
            scalar=gscale[:, 0:1], in1=x_sb[:, 0, h0:h0 + RB, :],
            op0=OP.mult, op1=OP.add)
        nc.vector.scalar_tensor_tensor(
            out=fin[:, 1], in0=ob_all[:, 1, h0:h0 + RB, :],
            scalar=gscale[:, 1:2], in1=x_sb[:, 1, h0:h0 + RB, :],
            op0=OP.mult, op1=OP.add)
        nc.sync.dma_start(
            out=y_d[:, h0:h0 + RB, :].rearrange("(kc p) h w -> p kc h w", p=128),
            in_=fin,
        )


def build_nc() -> bass.Bass:
    nc = bacc.Bacc()
    xh_d = nc.dram_tensor("xh", [C, H, W], F16, kind="ExternalInput")
    wqk_d = nc.dram_tensor("wqkT", [C, 128], F16, kind="ExternalInput")
    wv_d = nc.dram_tensor("wvT", [C, C], F16, kind="ExternalInput")
    w1_d = nc.dram_tensor("w1T", [C, HID], F32, kind="ExternalInput")
    w2_d = nc.dram_tensor("w2T", [HID, C], F32, kind="ExternalInput")
    gama_d = nc.dram_tensor("gama", [1, 1], F32, kind="ExternalInput")
    y_d = nc.dram_tensor("out", [C, H, W], F16, kind="ExternalOutput")

    with tile.TileContext(nc) as tc:
        with ExitStack() as ctx:
            _body(ctx, tc, xh_d[:, :, :], wqk_d[:, :],
                  wv_d[:, :], w1_d[:, :], w2_d[:, :], gama_d[:, :],
                  y_d[:, :, :])
    nc.compile()
    return nc


_NC_CACHE = {}


def _get_nc():
    if "nc" not in _NC_CACHE:
        _NC_CACHE["nc"] = build_nc()
    return _NC_CACHE["nc"]


def _make_in_maps(x, Wq, Wk, Wv, W1, W2, gama):
    wqkT = np.ascontiguousarray(
        np.concatenate([Wk, Wq], axis=0).T.astype(np.float16))
    wvT = np.ascontiguousarray(Wv.T.astype(np.float16))
    w1T = np.ascontiguousarray(W1.T.astype(np.float32))
    w2T = np.ascontiguousarray(W2.T.astype(np.float32))
    g = np.asarray(gama, dtype=np.float32).reshape(1, 1)
    maps = []
    for i in range(NCORES):
        maps.append({
            "xh": np.ascontiguousarray(x[i].astype(np.float16)),
            "wqkT": wqkT, "wvT": wvT, "w1T": w1T, "w2T": w2T, "gama": g,
        })
    return maps


def run(x, Wq, Wk, Wv, W1, W2, gama, trace=False):
    nc = _get_nc()
    in_maps = _make_in_maps(x, Wq, Wk, Wv, W1, W2, gama)
    res = run_bass_kernel_spmd(nc, in_maps, core_ids=list(range(NCORES)),
                               trace=trace)
    y = np.stack([res.results[i]["out"].astype(np.float32)
                  for i in range(NCORES)], axis=0)
    return y, res


def kernel(x, Wq, Wk, Wv, W1, W2, gama):
    x = np.asarray(x); Wq = np.asarray(Wq); Wk = np.asarray(Wk)
    Wv = np.asarray(Wv); W1 = np.asarray(W1); W2 = np.asarray(W2)
    gama = np.asarray(gama)
    y, _ = run(x, Wq, Wk, Wv, W1, W2, gama, trace=False)
    return y.astype(np.float32)


# revision 20
# speedup vs baseline: 1.1388x; 1.0812x over previous
"""ChannelRowAttention Trainium2 kernel (v3).

Full-input contract: kernel(**inputs) takes the complete (8,256,128,128) batch
plus weights, shards batch-wise across 8 NeuronCores (one image per core), and
returns the full (8,256,128,128) output.

Per-core plan (x_img = (256,128,128), fp16 on chip; residual path fp16):
  x resident in SBUF (fp16, 64KB/partition), loaded once in 8 chunks.

  pass 1, per 4-row block (software-pipelined one block ahead on PE):
    kq     = [Wk|Wq]^T . x_rows      (PE, M=128: psum parts 0:64=k, 64:128=q)
    kq -> SBUF fp16 (ACT)
    attT_r = k_r^T q_r               (PE, K=64; lhsT=k@base0, rhs=q@base64 via
                                      explicit tile_position=(0,0))
    exp on ACT (fp32 psum -> bf16 SBUF; no max-subtraction: |score|<40 and
    bf16 holds e^40)
    den    = ones128^T attT_e        (PE, M=128 -> den replicated across
                                      partitions in one matmul)
    inv    = 1/den                   (DVE approx reciprocal, psum -> SBUF)
    vT_r   = x_row^T . Wv^T          (PE, N=256 per row; one ACT copy)
    out_r  = vT^T . attT_e           (PE; UNNORMALIZED, fp32 psum)
    psum -> resident fp16 out via DVE scalar_tensor_tensor: multiplies by
    inv[(r,i)] (softmax normalization) in the same instruction; accum_out
    gives the per-channel sum stat. Running max stat on DVE (ping-pong).
  gate = sigmoid(W2.relu(W1.avg) + W2.relu(W1.max)): tiny fp32 PE matmuls
  pass 2, per block: final = out*(gama*gate[c]) + x -> DRAM fp16
    ch0 on DVE (scalar_tensor_tensor); ch1 on PE (diag(gscale) @ out,
    += ident @ x) with an ACT psum->SBUF copy, balancing engine load.
  (host casts the fp16 result back to fp32)
"""

import numpy as np
from contextlib import ExitStack

import concourse.bass as bass
from concourse import bacc
import concourse.tile as tile
from concourse import mybir
from concourse.bass_utils import run_bass_kernel_spmd

F32 = mybir.dt.float32
F16 = mybir.dt.float16
BF16 = mybir.dt.bfloat16

N, C, H, W = 8, 256, 128, 128
QK = 64
HID = 16          # SE hidden dim = C // 16
NCORES = 8
RB = 4            # rows per block
NBLK = H // RB    # 32
INV_HW = 1.0 / float(H * W)

AX = mybir.AxisListType
OP = mybir.AluOpType
AF = mybir.ActivationFunctionType

# toggles
USE_TP_HACK = False   # att matmul with lhsT@base0 / rhs@base64, tile_position=(0,0)


def _body(ctx: ExitStack, tc: "tile.TileContext", xh_d, wqk_d, wv_d,
          w1_d, w2_d, gama_d, id_d, y_d):
    nc = tc.nc

    const = ctx.enter_context(tc.tile_pool(name="const", bufs=1))
    stats = ctx.enter_context(tc.tile_pool(name="stats", bufs=1))
    xpool = ctx.enter_context(tc.tile_pool(name="xpool", bufs=1))
    opool = ctx.enter_context(tc.tile_pool(name="opool", bufs=1))
    work = ctx.enter_context(tc.tile_pool(name="work", bufs=2))
    finpool = ctx.enter_context(tc.tile_pool(name="fin", bufs=4))
    psK = ctx.enter_context(tc.tile_pool(name="psK", bufs=1, space="PSUM"))
    psT = ctx.enter_context(
        tc.tile_pool(name="psT", bufs=2 if USE_TP_HACK else 1, space="PSUM"))
    psD = ctx.enter_context(tc.tile_pool(name="psD", bufs=1, space="PSUM"))
    psV = ctx.enter_context(tc.tile_pool(name="psV", bufs=1, space="PSUM"))
    psO = ctx.enter_context(tc.tile_pool(name="psO", bufs=1, space="PSUM"))

    # ---- constants -------------------------------------------------------
    wqk_sb = const.tile([128, 2, 128], F16)
    nc.sync.dma_start(out=wqk_sb, in_=wqk_d[:, :].rearrange("(kc p) m -> p kc m", p=128))
    wv_sb = const.tile([128, 2, C], F16)
    nc.sync.dma_start(out=wv_sb, in_=wv_d[:, :].rearrange("(kc p) m -> p kc m", p=128))
    w1_sb = const.tile([128, 2, HID], F32)
    nc.sync.dma_start(out=w1_sb, in_=w1_d[:, :].rearrange("(kc p) m -> p kc m", p=128))
    w2_sb = const.tile([HID, 2, 128], F32)
    nc.sync.dma_start(out=w2_sb, in_=w2_d[:, :].rearrange("k (mc m) -> k mc m", m=128))
    gama_sb = const.tile([128, 1], F32)
    nc.sync.dma_start(out=gama_sb, in_=gama_d[:, :].to_broadcast([128, 1]))
    ident = const.tile([128, 128], F16)
    nc.sync.dma_start(out=ident, in_=id_d[:, :])
    ones_sb = const.tile([128, 128], BF16)
    nc.vector.memset(ones_sb, 1.0)
    gscale = const.tile([128, 2], F32)      # gama * sigmoid(gate), filled later
    diag1 = const.tile([128, 128], F16)     # diag(gscale[:,1]), filled later

    # ---- resident image + attention output ------------------------------
    x_sb = xpool.tile([128, 2, H, W], F16)
    CHUNK = 16
    for i in range(H // CHUNK):
        nc.sync.dma_start(
            out=x_sb[:, :, i * CHUNK:(i + 1) * CHUNK, :],
            in_=xh_d[:, i * CHUNK:(i + 1) * CHUNK, :]
                .rearrange("(kc p) h w -> p kc h w", p=128),
        )
    ob_all = opool.tile([128, 2, H, W], F16)

    sums_acc = stats.tile([128, 2, NBLK], F32)
    nc.vector.memset(sums_acc, 0.0)
    mxa = stats.tile([128, 2, RB, W], F16)
    nc.vector.memset(mxa, -60000.0)
    mxb = stats.tile([128, 2, RB, W], F16)

    # ---- pass 1 (PE pipelined one block ahead) ---------------------------
    kq_sbs = [None] * NBLK
    k2_sbs = [None] * NBLK

    def stage_kq(b):
        h0 = b * RB
        xr = x_sb[:, :, h0:h0 + RB, :]
        kq_ps = psK.tile([128, RB, W], F32, tag="kq")
        for kc in (0, 1):
            nc.tensor.matmul(
                out=kq_ps[:, :, :].rearrange("p r w -> p (r w)"),
                lhsT=wqk_sb[:, kc, :],
                rhs=xr[:, kc, :, :].rearrange("p r w -> p (r w)"),
                start=(kc == 0), stop=(kc == 1),
            )
        kq_sb = work.tile([128, RB, W], F16, tag="kq_sb")
        nc.scalar.copy(out=kq_sb, in_=kq_ps)
        kq_sbs[b] = kq_sb
        if not USE_TP_HACK:
            # replicate k (parts 0:64) to parts 64:128 so att operands share
            # a base partition
            k2_sb = work.tile([128, RB, W], F16, tag="k2_sb")
            k2_ps = psK.tile([128, RB, W], F32, tag="kq2")
            for kc in (0, 1):
                nc.tensor.matmul(
                    out=k2_ps[64:128, :, :].rearrange("p r w -> p (r w)"),
                    lhsT=wqk_sb[:, kc, 0:64],
                    rhs=xr[:, kc, :, :].rearrange("p r w -> p (r w)"),
                    start=(kc == 0), stop=(kc == 1),
                )
            nc.scalar.copy(out=k2_sb[64:128, :, :], in_=k2_ps[64:128, :, :])
            k2_sbs[b] = k2_sb

    def stage_main(b):
        h0 = b * RB
        kq_sb = kq_sbs[b]

        # attT[j, i] per row (K=64)
        attT_ps = psT.tile([128, RB, W], F32, tag="attT")
        for r in range(RB):
            if USE_TP_HACK:
                nc.tensor.matmul(
                    out=attT_ps[:, r, :],
                    lhsT=kq_sb[0:64, r, :],
                    rhs=kq_sb[64:128, r, :],
                    start=True, stop=True, tile_position=(0, 0),
                )
            else:
                nc.tensor.matmul(
                    out=attT_ps[:, r, :],
                    lhsT=k2_sbs[b][64:128, r, :],
                    rhs=kq_sb[64:128, r, :],
                    start=True, stop=True,
                )
        attT_e = work.tile([128, RB, W], BF16, tag="attT_e")
        nc.scalar.activation(out=attT_e, in_=attT_ps, func=AF.Exp)

        # vT per row (w on partitions, c on free)
        vt_ps = psV.tile([128, RB, C], F32, tag="vt")
        for r in range(RB):
            for kc in (0, 1):
                nc.tensor.matmul(
                    out=vt_ps[:, r, :],
                    lhsT=x_sb[:, kc, h0 + r, :],
                    rhs=wv_sb[:, kc, :],
                    start=(kc == 0), stop=(kc == 1),
                )

        # softmax denominator, replicated across partitions in one matmul
        den_ps = psD.tile([128, RB * W], F32, tag="den")
        nc.tensor.matmul(
            out=den_ps,
            lhsT=ones_sb,
            rhs=attT_e[:, :, :].rearrange("p r w -> p (r w)"),
            start=True, stop=True,
        )
        inv_b = work.tile([128, RB, W], F32, tag="inv_b")
        nc.vector.reciprocal_approx_fast(
            out=inv_b[:, :, :].rearrange("p r w -> p (r w)"), in_=den_ps)

        vt_sb = work.tile([128, RB, C], BF16, tag="vt_sb")
        nc.scalar.copy(out=vt_sb, in_=vt_ps)

        # out = vT^T @ attT_e -> (c, i), unnormalized fp32 in psum
        out_ps = psO.tile([128, 2, RB, W], F32, tag="out")
        for r in range(RB):
            for ch in (0, 1):
                nc.tensor.matmul(
                    out=out_ps[:, ch, r, :],
                    lhsT=vt_sb[:, r, 128 * ch:128 * (ch + 1)],
                    rhs=attT_e[:, r, :],
                    start=True, stop=True,
                )

        # psum -> resident fp16, normalizing by inv[(r,i)]; accum -> sums
        for ch in (0, 1):
            nc.vector.scalar_tensor_tensor(
                out=ob_all[:, ch, h0:h0 + RB, :],
                in0=out_ps[:, ch], scalar=1.0, in1=inv_b,
                op0=OP.mult, op1=OP.mult,
                accum_out=sums_acc[:, ch, b:b + 1])
        # running max stat (DVE), ping-pong accumulators
        src, dst = (mxa, mxb) if b % 2 == 0 else (mxb, mxa)
        nc.vector.tensor_tensor(
            out=dst, in0=src, in1=ob_all[:, :, h0:h0 + RB, :], op=OP.max)

    for b in range(NBLK + 1):
        if b < NBLK:
            stage_kq(b)
        if b >= 1:
            stage_main(b - 1)

    # ---- gate ------------------------------------------------------------
    mxfin = mxa if NBLK % 2 == 0 else mxb
    mx = stats.tile([128, 2], F32)
    nc.vector.tensor_reduce(out=mx, in_=mxfin, axis=AX.XY, op=OP.max)

    mlp_in = stats.tile([128, 2, 2], F32)
    sums = stats.tile([128, 2], F32)
    nc.vector.tensor_reduce(out=sums, in_=sums_acc, axis=AX.X, op=OP.add)
    nc.vector.tensor_scalar_mul(out=mlp_in[:, :, 0], in0=sums, scalar1=INV_HW)
    nc.vector.tensor_copy(out=mlp_in[:, :, 1], in_=mx)

    h_ps = psD.tile([HID, 2], F32, tag="den")
    for kc in (0, 1):
        nc.tensor.matmul(
            out=h_ps,
            lhsT=w1_sb[:, kc, :],
            rhs=mlp_in[:, kc, :],
            start=(kc == 0), stop=(kc == 1),
        )
    hr = stats.tile([HID, 2], F32)
    nc.vector.tensor_scalar_max(out=hr, in0=h_ps, scalar1=0.0)
    g_ps = psD.tile([128, 2, 2], F32, tag="den")
    for mc in (0, 1):
        nc.tensor.matmul(
            out=g_ps[:, mc, :],
            lhsT=w2_sb[:, mc, :],
            rhs=hr,
            start=True, stop=True,
        )
    zt = stats.tile([128, 2], F32)
    nc.vector.tensor_reduce(out=zt, in_=g_ps, axis=AX.X, op=OP.add)
    th = stats.tile([128, 2], F32)
    nc.scalar.activation(out=th, in_=zt, func=AF.Tanh, scale=0.5)
    u = stats.tile([128, 2], F32)
    nc.vector.tensor_scalar_add(out=u, in0=th, scalar1=1.0)
    # gscale = gama * sigmoid(z) = gama * 0.5 * (1 + tanh(z/2))
    nc.vector.tensor_scalar(
        out=gscale, in0=u, scalar1=gama_sb, scalar2=0.5, op0=OP.mult, op1=OP.mult)
    # diag(gscale[:,1]) for the PE-side pass-2 channel
    nc.vector.tensor_scalar_mul(out=diag1, in0=ident, scalar1=gscale[:, 1:2])

    # ---- pass 2: final = out*gscale[c] + x -> DRAM (fp16) ----------------
    # ch0 on DVE, ch1 on PE+ACT to balance engine load
    for b in range(NBLK):
        h0 = b * RB
        fin = finpool.tile([128, 2, RB, W], F16, tag="fin")
        nc.vector.scalar_tensor_tensor(
            out=fin[:, 0], in0=ob_all[:, 0, h0:h0 + RB, :],
            scalar=gscale[:, 0:1], in1=x_sb[:, 0, h0:h0 + RB, :],
            op0=OP.mult, op1=OP.add)
        p2_ps = psD.tile([128, RB * W], F32, tag="den")
        nc.tensor.matmul(
            out=p2_ps,
            lhsT=diag1,
            rhs=ob_all[:, 1, h0:h0 + RB, :].rearrange("p r w -> p (r w)"),
            start=True, stop=False,
        )
        nc.tensor.matmul(
            out=p2_ps,
            lhsT=ident,
            rhs=x_sb[:, 1, h0:h0 + RB, :].rearrange("p r w -> p (r w)"),
            start=False, stop=True,
        )
        nc.scalar.copy(
            out=fin[:, 1].rearrange("p r w -> p (r w)"), in_=p2_ps)
        nc.sync.dma_start(
            out=y_d[:, h0:h0 + RB, :].rearrange("(kc p) h w -> p kc h w", p=128),
            in_=fin,
        )


def build_nc() -> bass.Bass:
    nc = bacc.Bacc()
    xh_d = nc.dram_tensor("xh", [C, H, W], F16, kind="ExternalInput")
    wqk_d = nc.dram_tensor("wqkT", [C, 128], F16, kind="ExternalInput")
    wv_d = nc.dram_tensor("wvT", [C, C], F16, kind="ExternalInput")
    w1_d = nc.dram_tensor("w1T", [C, HID], F32, kind="ExternalInput")
    w2_d = nc.dram_tensor("w2T", [HID, C], F32, kind="ExternalInput")
    gama_d = nc.dram_tensor("gama", [1, 1], F32, kind="ExternalInput")
    id_d = nc.dram_tensor("ident", [128, 128], F16, kind="ExternalInput")
    y_d = nc.dram_tensor("out", [C, H, W], F16, kind="ExternalOutput")

    with tile.TileContext(nc) as tc:
        with ExitStack() as ctx:
            _body(ctx, tc, xh_d[:, :, :], wqk_d[:, :],
                  wv_d[:, :], w1_d[:, :], w2_d[:, :], gama_d[:, :],
                  id_d[:, :], y_d[:, :, :])
    nc.compile()
    return nc


_NC_CACHE = {}


def _get_nc():
    if "nc" not in _NC_CACHE:
        _NC_CACHE["nc"] = build_nc()
    return _NC_CACHE["nc"]


def _make_in_maps(x, Wq, Wk, Wv, W1, W2, gama):
    wqkT = np.ascontiguousarray(
        np.concatenate([Wk, Wq], axis=0).T.astype(np.float16))
    wvT = np.ascontiguousarray(Wv.T.astype(np.float16))
    w1T = np.ascontiguousarray(W1.T.astype(np.float32))
    w2T = np.ascontiguousarray(W2.T.astype(np.float32))
    g = np.asarray(gama, dtype=np.float32).reshape(1, 1)
    ident = np.eye(128, dtype=np.float16)
    maps = []
    for i in range(NCORES):
        maps.append({
            "xh": np.ascontiguousarray(x[i].astype(np.float16)),
            "wqkT": wqkT, "wvT": wvT, "w1T": w1T, "w2T": w2T, "gama": g,
            "ident": ident,
        })
    return maps


def run(x, Wq, Wk, Wv, W1, W2, gama, trace=False):
    nc = _get_nc()
    in_maps = _make_in_maps(x, Wq, Wk, Wv, W1, W2, gama)
    res = run_bass_kernel_spmd(nc, in_maps, core_ids=list(range(NCORES)),
                               trace=trace)
    y = np.stack([res.results[i]["out"].astype(np.float32)
                  for i in range(NCORES)], axis=0)
    return y, res


def kernel(x, Wq, Wk, Wv, W1, W2, gama):
    x = np.asarray(x); Wq = np.asarray(Wq); Wk = np.asarray(Wk)
    Wv = np.asarray(Wv); W1 = np.asarray(W1); W2 = np.asarray(W2)
    gama = np.asarray(gama)
    y, _ = run(x, Wq, Wk, Wv, W1, W2, gama, trace=False)
    return y.astype(np.float32)


# revision 21
# speedup vs baseline: 1.2541x; 1.1013x over previous
"""ChannelRowAttention Trainium2 kernel (v4).

Full-input contract: kernel(**inputs) takes the complete (8,256,128,128) batch
plus weights, shards batch-wise across 8 NeuronCores (one image per core), and
returns the full (8,256,128,128) output.

Per-core plan (x_img = (256,128,128), fp16 on chip; residual path fp16):
  x resident in SBUF (fp16, 64KB/partition), loaded once in 8 chunks.

  pass 1, per 4-row block, 3-deep software pipeline (kq | mid | out stages)
  to keep the PE continuously busy (PE clock ramps 1.2->2.4GHz only under
  sustained use):
    kq     = [Wk|Wq]^T . x_rows      (PE, M=128: psum parts 0:64=k, 64:128=q)
    kq -> SBUF fp16 (ACT); k replicated to parts 64:128 via SBUF->SBUF DMA
    (issued a full block ahead of its use, so DMA latency is hidden)
    attT_r = k_r^T q_r               (PE, K=64 at base 64) - attT directly,
                                     no PE transposes anywhere
    exp on ACT (fp32 psum -> bf16 SBUF; no max-subtraction: |score|<40 and
    bf16 holds e^40)
    den    = ones128^T attT_e        (PE, M=128 -> den replicated across all
                                      psum partitions in one matmul)
    inv    = 1/den                   (DVE approx reciprocal, psum -> SBUF)
    vT_r   = x_row^T . Wv^T          (PE, N=256 per row; one ACT copy)
    out_r  = vT^T . attT_e           (PE; UNNORMALIZED, fp32 psum)
    psum -> resident fp16 out via DVE scalar_tensor_tensor: multiplies by
    inv[(r,i)] (softmax normalization) in the same instruction; accum_out
    gives the per-channel sum stat. Running max stat on DVE (ping-pong).
  gate = sigmoid(W2.relu(W1.avg) + W2.relu(W1.max)): tiny fp32 PE matmuls
  pass 2, per block: final = out*(gama*gate[c]) + x -> DRAM fp16
    ch0 on DVE (scalar_tensor_tensor); ch1 on PE (diag(gscale) @ out
    += ident @ x) + ACT psum->SBUF copy, balancing engine load.
  (host casts the fp16 result back to fp32)
"""

import numpy as np
from contextlib import ExitStack

import concourse.bass as bass
from concourse import bacc
import concourse.tile as tile
from concourse import mybir
from concourse.bass_utils import run_bass_kernel_spmd

F32 = mybir.dt.float32
F16 = mybir.dt.float16
BF16 = mybir.dt.bfloat16

N, C, H, W = 8, 256, 128, 128
QK = 64
HID = 16          # SE hidden dim = C // 16
NCORES = 8
RB = 4            # rows per block
NBLK = H // RB    # 32
INV_HW = 1.0 / float(H * W)

AX = mybir.AxisListType
OP = mybir.AluOpType
AF = mybir.ActivationFunctionType


def _body(ctx: ExitStack, tc: "tile.TileContext", xh_d, wqk_d, wv_d,
          w1_d, w2_d, gama_d, id_d, y_d):
    nc = tc.nc

    const = ctx.enter_context(tc.tile_pool(name="const", bufs=1))
    stats = ctx.enter_context(tc.tile_pool(name="stats", bufs=1))
    xpool = ctx.enter_context(tc.tile_pool(name="xpool", bufs=1))
    opool = ctx.enter_context(tc.tile_pool(name="opool", bufs=1))
    work = ctx.enter_context(tc.tile_pool(name="work", bufs=2))
    finpool = ctx.enter_context(tc.tile_pool(name="fin", bufs=4))
    # PSUM budget (8 banks): kq 1 | attT/den shared tag 2 | vt 2 | out 2 | p2 1
    psK = ctx.enter_context(tc.tile_pool(name="psK", bufs=1, space="PSUM"))
    psT = ctx.enter_context(tc.tile_pool(name="psT", bufs=2, space="PSUM"))
    psV = ctx.enter_context(tc.tile_pool(name="psV", bufs=1, space="PSUM"))
    psO = ctx.enter_context(tc.tile_pool(name="psO", bufs=1, space="PSUM"))
    psP2 = ctx.enter_context(tc.tile_pool(name="psP2", bufs=1, space="PSUM"))

    # ---- constants -------------------------------------------------------
    wqk_sb = const.tile([128, 2, 128], F16)
    nc.sync.dma_start(out=wqk_sb, in_=wqk_d[:, :].rearrange("(kc p) m -> p kc m", p=128))
    wv_sb = const.tile([128, 2, C], F16)
    nc.sync.dma_start(out=wv_sb, in_=wv_d[:, :].rearrange("(kc p) m -> p kc m", p=128))
    w1_sb = const.tile([128, 2, HID], F32)
    nc.sync.dma_start(out=w1_sb, in_=w1_d[:, :].rearrange("(kc p) m -> p kc m", p=128))
    w2_sb = const.tile([HID, 2, 128], F32)
    nc.sync.dma_start(out=w2_sb, in_=w2_d[:, :].rearrange("k (mc m) -> k mc m", m=128))
    gama_sb = const.tile([128, 1], F32)
    nc.sync.dma_start(out=gama_sb, in_=gama_d[:, :].to_broadcast([128, 1]))
    ident = const.tile([128, 128], F16)
    nc.sync.dma_start(out=ident, in_=id_d[:, :])
    ones_sb = const.tile([128, 128], BF16)
    nc.vector.memset(ones_sb, 1.0)
    gscale = const.tile([128, 2], F32)      # gama * sigmoid(gate), filled later
    diag1 = const.tile([128, 128], F16)     # diag(gscale[:,1]), filled later

    # ---- resident image + attention output ------------------------------
    x_sb = xpool.tile([128, 2, H, W], F16)
    CHUNK = 16
    for i in range(H // CHUNK):
        nc.sync.dma_start(
            out=x_sb[:, :, i * CHUNK:(i + 1) * CHUNK, :],
            in_=xh_d[:, i * CHUNK:(i + 1) * CHUNK, :]
                .rearrange("(kc p) h w -> p kc h w", p=128),
        )
    ob_all = opool.tile([128, 2, H, W], F16)

    sums_acc = stats.tile([128, 2, NBLK], F32)
    nc.vector.memset(sums_acc, 0.0)
    mxa = stats.tile([128, 2, RB, W], F16)
    nc.vector.memset(mxa, -60000.0)
    mxb = stats.tile([128, 2, RB, W], F16)

    # ---- pass 1: 3-deep pipeline -----------------------------------------
    kq_sbs = [None] * NBLK
    k2_sbs = [None] * NBLK
    attT_es = [None] * NBLK
    inv_bs = [None] * NBLK
    vt_sbs = [None] * NBLK
    out_pss = [None] * NBLK

    def stage_kq(b):
        h0 = b * RB
        xr = x_sb[:, :, h0:h0 + RB, :]
        kq_ps = psK.tile([128, RB, W], F32, tag="kq")
        for kc in (0, 1):
            nc.tensor.matmul(
                out=kq_ps[:, :, :].rearrange("p r w -> p (r w)"),
                lhsT=wqk_sb[:, kc, :],
                rhs=xr[:, kc, :, :].rearrange("p r w -> p (r w)"),
                start=(kc == 0), stop=(kc == 1),
            )
        kq_sb = work.tile([128, RB, W], F16, tag="kq_sb")
        nc.scalar.copy(out=kq_sb, in_=kq_ps)
        # replicate k (parts 0:64) to parts 64:128 so att operands share a
        # base partition; the DMA is issued a full block ahead of its use
        k2_sb = work.tile([128, RB, W], F16, tag="k2_sb")
        nc.sync.dma_start(out=k2_sb[64:128, :, :], in_=kq_sb[0:64, :, :])
        kq_sbs[b] = kq_sb
        k2_sbs[b] = k2_sb

    def stage_mid(b):
        h0 = b * RB
        kq_sb, k2_sb = kq_sbs[b], k2_sbs[b]

        # attT[j, i] per row (K=64 at base partition 64)
        attT_ps = psT.tile([128, RB, W], F32, tag="attT")
        for r in range(RB):
            nc.tensor.matmul(
                out=attT_ps[:, r, :],
                lhsT=k2_sb[64:128, r, :],
                rhs=kq_sb[64:128, r, :],
                start=True, stop=True,
            )
        attT_e = work.tile([128, RB, W], BF16, tag="attT_e")
        nc.scalar.activation(out=attT_e, in_=attT_ps, func=AF.Exp)
        attT_es[b] = attT_e

        # vT per row (w on partitions, c on free)
        vt_ps = psV.tile([128, RB, C], F32, tag="vt")
        for r in range(RB):
            for kc in (0, 1):
                nc.tensor.matmul(
                    out=vt_ps[:, r, :],
                    lhsT=x_sb[:, kc, h0 + r, :],
                    rhs=wv_sb[:, kc, :],
                    start=(kc == 0), stop=(kc == 1),
                )
        vt_sb = work.tile([128, RB, C], BF16, tag="vt_sb")
        nc.scalar.copy(out=vt_sb, in_=vt_ps)
        vt_sbs[b] = vt_sb

        # softmax denominator, replicated across partitions in one matmul;
        # shares the attT psum tag (ping-pong within the 2 bufs)
        den_ps = psT.tile([128, RB * W], F32, tag="attT")
        nc.tensor.matmul(
            out=den_ps,
            lhsT=ones_sb,
            rhs=attT_e[:, :, :].rearrange("p r w -> p (r w)"),
            start=True, stop=True,
        )
        inv_b = work.tile([128, RB, W], F32, tag="inv_b")
        nc.vector.reciprocal_approx_fast(
            out=inv_b[:, :, :].rearrange("p r w -> p (r w)"), in_=den_ps)
        inv_bs[b] = inv_b

    def stage_out(b):
        h0 = b * RB
        attT_e, vt_sb, inv_b = attT_es[b], vt_sbs[b], inv_bs[b]

        # out = vT^T @ attT_e -> (c, i), unnormalized fp32 in psum
        out_ps = psO.tile([128, 2, RB, W], F32, tag="out")
        for r in range(RB):
            for ch in (0, 1):
                nc.tensor.matmul(
                    out=out_ps[:, ch, r, :],
                    lhsT=vt_sb[:, r, 128 * ch:128 * (ch + 1)],
                    rhs=attT_e[:, r, :],
                    start=True, stop=True,
                )
        # psum -> resident fp16, normalizing by inv[(r,i)]; accum -> sums
        for ch in (0, 1):
            nc.vector.scalar_tensor_tensor(
                out=ob_all[:, ch, h0:h0 + RB, :],
                in0=out_ps[:, ch], scalar=1.0, in1=inv_b,
                op0=OP.mult, op1=OP.mult,
                accum_out=sums_acc[:, ch, b:b + 1])
        # running max stat (DVE), ping-pong accumulators
        src, dst = (mxa, mxb) if b % 2 == 0 else (mxb, mxa)
        nc.vector.tensor_tensor(
            out=dst, in0=src, in1=ob_all[:, :, h0:h0 + RB, :], op=OP.max)

    for i in range(NBLK + 2):
        if i >= 2:
            stage_out(i - 2)
        if i < NBLK:
            stage_kq(i)
        if 1 <= i <= NBLK:
            stage_mid(i - 1)

    # ---- gate ------------------------------------------------------------
    mxfin = mxa if NBLK % 2 == 0 else mxb
    mx = stats.tile([128, 2], F32)
    nc.vector.tensor_reduce(out=mx, in_=mxfin, axis=AX.XY, op=OP.max)

    mlp_in = stats.tile([128, 2, 2], F32)
    sums = stats.tile([128, 2], F32)
    nc.vector.tensor_reduce(out=sums, in_=sums_acc, axis=AX.X, op=OP.add)
    nc.vector.tensor_scalar_mul(out=mlp_in[:, :, 0], in0=sums, scalar1=INV_HW)
    nc.vector.tensor_copy(out=mlp_in[:, :, 1], in_=mx)

    h_ps = psP2.tile([HID, 2], F32, tag="p2")
    for kc in (0, 1):
        nc.tensor.matmul(
            out=h_ps,
            lhsT=w1_sb[:, kc, :],
            rhs=mlp_in[:, kc, :],
            start=(kc == 0), stop=(kc == 1),
        )
    hr = stats.tile([HID, 2], F32)
    nc.vector.tensor_scalar_max(out=hr, in0=h_ps, scalar1=0.0)
    g_ps = psP2.tile([128, 2, 2], F32, tag="p2")
    for mc in (0, 1):
        nc.tensor.matmul(
            out=g_ps[:, mc, :],
            lhsT=w2_sb[:, mc, :],
            rhs=hr,
            start=True, stop=True,
        )
    zt = stats.tile([128, 2], F32)
    nc.vector.tensor_reduce(out=zt, in_=g_ps, axis=AX.X, op=OP.add)
    th = stats.tile([128, 2], F32)
    nc.scalar.activation(out=th, in_=zt, func=AF.Tanh, scale=0.5)
    u = stats.tile([128, 2], F32)
    nc.vector.tensor_scalar_add(out=u, in0=th, scalar1=1.0)
    # gscale = gama * sigmoid(z) = gama * 0.5 * (1 + tanh(z/2))
    nc.vector.tensor_scalar(
        out=gscale, in0=u, scalar1=gama_sb, scalar2=0.5, op0=OP.mult, op1=OP.mult)
    # diag(gscale[:,1]) for the PE-side pass-2 channel
    nc.vector.tensor_scalar_mul(out=diag1, in0=ident, scalar1=gscale[:, 1:2])

    # ---- pass 2: final = out*gscale[c] + x -> DRAM (fp16) ----------------
    # ch0 on DVE, ch1 on PE+ACT; psum alternates psP2 / the psT banks
    for b in range(NBLK):
        h0 = b * RB
        fin = finpool.tile([128, 2, RB, W], F16, tag="fin")
        nc.vector.scalar_tensor_tensor(
            out=fin[:, 0], in0=ob_all[:, 0, h0:h0 + RB, :],
            scalar=gscale[:, 0:1], in1=x_sb[:, 0, h0:h0 + RB, :],
            op0=OP.mult, op1=OP.add)
        if b % 2 == 0:
            p2_ps = psP2.tile([128, RB * W], F32, tag="p2")
        else:
            p2_ps = psT.tile([128, RB * W], F32, tag="attT")
        nc.tensor.matmul(
            out=p2_ps,
            lhsT=diag1,
            rhs=ob_all[:, 1, h0:h0 + RB, :].rearrange("p r w -> p (r w)"),
            start=True, stop=False,
        )
        nc.tensor.matmul(
            out=p2_ps,
            lhsT=ident,
            rhs=x_sb[:, 1, h0:h0 + RB, :].rearrange("p r w -> p (r w)"),
            start=False, stop=True,
        )
        nc.scalar.copy(
            out=fin[:, 1].rearrange("p r w -> p (r w)"), in_=p2_ps)
        nc.sync.dma_start(
            out=y_d[:, h0:h0 + RB, :].rearrange("(kc p) h w -> p kc h w", p=128),
            in_=fin,
        )


def build_nc() -> bass.Bass:
    nc = bacc.Bacc()
    xh_d = nc.dram_tensor("xh", [C, H, W], F16, kind="ExternalInput")
    wqk_d = nc.dram_tensor("wqkT", [C, 128], F16, kind="ExternalInput")
    wv_d = nc.dram_tensor("wvT", [C, C], F16, kind="ExternalInput")
    w1_d = nc.dram_tensor("w1T", [C, HID], F32, kind="ExternalInput")
    w2_d = nc.dram_tensor("w2T", [HID, C], F32, kind="ExternalInput")
    gama_d = nc.dram_tensor("gama", [1, 1], F32, kind="ExternalInput")
    id_d = nc.dram_tensor("ident", [128, 128], F16, kind="ExternalInput")
    y_d = nc.dram_tensor("out", [C, H, W], F16, kind="ExternalOutput")

    with tile.TileContext(nc) as tc:
        with ExitStack() as ctx:
            _body(ctx, tc, xh_d[:, :, :], wqk_d[:, :],
                  wv_d[:, :], w1_d[:, :], w2_d[:, :], gama_d[:, :],
                  id_d[:, :], y_d[:, :, :])
    nc.compile()
    return nc


_NC_CACHE = {}


def _get_nc():
    if "nc" not in _NC_CACHE:
        _NC_CACHE["nc"] = build_nc()
    return _NC_CACHE["nc"]


def _make_in_maps(x, Wq, Wk, Wv, W1, W2, gama):
    wqkT = np.ascontiguousarray(
        np.concatenate([Wk, Wq], axis=0).T.astype(np.float16))
    wvT = np.ascontiguousarray(Wv.T.astype(np.float16))
    w1T = np.ascontiguousarray(W1.T.astype(np.float32))
    w2T = np.ascontiguousarray(W2.T.astype(np.float32))
    g = np.asarray(gama, dtype=np.float32).reshape(1, 1)
    ident = np.eye(128, dtype=np.float16)
    maps = []
    for i in range(NCORES):
        maps.append({
            "xh": np.ascontiguousarray(x[i].astype(np.float16)),
            "wqkT": wqkT, "wvT": wvT, "w1T": w1T, "w2T": w2T, "gama": g,
            "ident": ident,
        })
    return maps


def run(x, Wq, Wk, Wv, W1, W2, gama, trace=False):
    nc = _get_nc()
    in_maps = _make_in_maps(x, Wq, Wk, Wv, W1, W2, gama)
    res = run_bass_kernel_spmd(nc, in_maps, core_ids=list(range(NCORES)),
                               trace=trace)
    y = np.stack([res.results[i]["out"].astype(np.float32)
                  for i in range(NCORES)], axis=0)
    return y, res


def kernel(x, Wq, Wk, Wv, W1, W2, gama):
    x = np.asarray(x); Wq = np.asarray(Wq); Wk = np.asarray(Wk)
    Wv = np.asarray(Wv); W1 = np.asarray(W1); W2 = np.asarray(W2)
    gama = np.asarray(gama)
    y, _ = run(x, Wq, Wk, Wv, W1, W2, gama, trace=False)
    return y.astype(np.float32)
